# revision 1
# baseline (speedup 1.0000x reference)
"""Trainium2 Bass kernel for nn_GroupFeatureBuilder (segment_reduce).

Strategy: shard the M=4096 groups across 8 cores (512 groups each).
Replace all gathers with dense matmuls against a host-built multiplicity
matrix C[m, n] = (# occurrences of robot n in group m):

  E      = C @ A            (A = attn_rr, bf16)       -> per-group attn rows
  t1[m]  = <E[m], C[m]>     = sum_{i,j} A[g_i, g_j]
  t2[m]  = <C[m]^2, diagA>  = sum over equal pairs
  t3[m]  = <E[m], mem[m]>   (mem = min(C,1), the unique-membership mask)
  esum[m]= sum_n E[m,n]     = rows.sum
  a_in   = (t1 - t2) / max(256 - sum C^2, 1)
  a_out  = (esum - t3) / (16 * (2048 - sum mem))
  HR     = C @ [h | attn_ro]  -> h_g (cols 0:256), a_obs (rowsum of 256:320)
  h_glob = ones^T @ h / 2048
  ex_dist/ex_clr: host-gathered (512,16) slots, device mean/min reduce.
"""

import numpy as np
import ml_dtypes

import concourse.bass as bass
import concourse.bacc as bacc
import concourse.tile as tile
import concourse.mybir as mybir
from concourse.bass_utils import run_bass_kernel_spmd

BF16 = ml_dtypes.bfloat16

N = 2048       # robots
D = 256        # embed
M = 4096       # groups
K = 16         # group size
NOBS = 64
NCORES = 8
MLOC = M // NCORES     # 512 groups per core
MCH = MLOC // 128      # 4 m-chunks
KCH = N // 128         # 16 contraction chunks
NCH = N // 512         # 4 column chunks of A
HRW = D + NOBS         # 320 columns of the HR rhs
FOUT = 2 * D + 6       # 518 output features

f32 = mybir.dt.float32
bf16 = mybir.dt.bfloat16
OP = mybir.AluOpType
AX = mybir.AxisListType
ACT = mybir.ActivationFunctionType

_NC_CACHE = {}


def _build_nc():
    nc = bacc.Bacc("TRN2", target_bir_lowering=False, debug=False,
                   num_devices=NCORES)

    a_d = nc.declare_dram_parameter("a_bf", [N, N], bf16, isOutput=False)
    hr_d = nc.declare_dram_parameter("hr_bf", [N, HRW], bf16, isOutput=False)
    ct_d = nc.declare_dram_parameter("ct_bf", [N, MLOC], bf16, isOutput=False)
    cm_d = nc.declare_dram_parameter("cm_bf", [MLOC, N], bf16, isOutput=False)
    diag_d = nc.declare_dram_parameter("diag_bf", [1, N], bf16, isOutput=False)
    cg_d = nc.declare_dram_parameter("cg", [MLOC, K], f32, isOutput=False)
    dg_d = nc.declare_dram_parameter("dg", [MLOC, K], f32, isOutput=False)
    out_d = nc.declare_dram_parameter("out", [MLOC, FOUT], f32, isOutput=True)

    with tile.TileContext(nc) as tc:
        with (
            tc.tile_pool(name="res", bufs=1) as res,
            tc.tile_pool(name="apool", bufs=1) as apool,
            tc.tile_pool(name="junk", bufs=3) as junkp,
            tc.tile_pool(name="stats", bufs=1) as statp,
            tc.tile_pool(name="psum_e", bufs=5, space="PSUM") as pe_pool,
            tc.tile_pool(name="psum_hr", bufs=2, space="PSUM") as phr_pool,
            tc.tile_pool(name="psum_g", bufs=1, space="PSUM") as pg_pool,
        ):
            # ---- resident loads ----
            a_t = []
            for k in range(KCH):
                t = apool.tile([128, N], bf16, tag=f"a{k}")
                nc.sync.dma_start(out=t[:], in_=a_d[k * 128:(k + 1) * 128, :])
                a_t.append(t)
            hr_t = []
            for k in range(KCH):
                t = res.tile([128, HRW], bf16, tag=f"hr{k}")
                nc.sync.dma_start(out=t[:], in_=hr_d[k * 128:(k + 1) * 128, :])
                hr_t.append(t)
            ct_t = []
            for k in range(KCH):
                t = res.tile([128, MLOC], bf16, tag=f"ct{k}")
                nc.sync.dma_start(out=t[:], in_=ct_d[k * 128:(k + 1) * 128, :])
                ct_t.append(t)
            cm_t = []
            for m in range(MCH):
                t = res.tile([128, N], bf16, tag=f"cm{m}")
                nc.sync.dma_start(out=t[:], in_=cm_d[m * 128:(m + 1) * 128, :])
                cm_t.append(t)
            cg_t = []
            dg_t = []
            for m in range(MCH):
                t = res.tile([128, K], f32, tag=f"cg{m}")
                nc.sync.dma_start(out=t[:], in_=cg_d[m * 128:(m + 1) * 128, :])
                cg_t.append(t)
                t = res.tile([128, K], f32, tag=f"dg{m}")
                nc.sync.dma_start(out=t[:], in_=dg_d[m * 128:(m + 1) * 128, :])
                dg_t.append(t)

            diag_row = res.tile([1, N], bf16, tag="diag_row")
            nc.sync.dma_start(out=diag_row[:], in_=diag_d[:, :])

            ones_t = res.tile([128, 1], bf16, tag="ones")
            nc.vector.memset(ones_t[:], 1.0)
            ones_row = res.tile([1, 128], bf16, tag="ones_row")
            nc.vector.memset(ones_row[:], 1.0)

            # broadcast diag across partitions via K=1 outer-product matmuls
            diag_b = res.tile([128, N], bf16, tag="diag_b")
            for n in range(NCH):
                pb = pe_pool.tile([128, 512], f32, tag="pe", name=f"pbd{n}")
                nc.tensor.matmul(pb[:], ones_row[:1],
                                 diag_row[:1, n * 512:(n + 1) * 512])
                nc.scalar.activation(diag_b[:, n * 512:(n + 1) * 512], pb[:],
                                     ACT.Copy)

            # ---- h_glob = mean_n h[n, :] via ones^T @ h ----
            pg = pg_pool.tile([1, D], f32)
            for k in range(KCH):
                nc.tensor.matmul(pg[:], ones_t[:], hr_t[k][:, 0:D],
                                 start=(k == 0), stop=(k == KCH - 1))
            hglob_row = res.tile([1, D], bf16, tag="hglob_row")
            nc.scalar.activation(hglob_row[:], pg[:], ACT.Copy, scale=1.0 / N)
            hglob_b = res.tile([128, D], f32, tag="hglob_b")
            pgb = pg_pool.tile([128, D], f32, name="pgb", tag="pg")
            nc.tensor.matmul(pgb[:], ones_row[:1], hglob_row[:1])
            nc.scalar.activation(hglob_b[:], pgb[:], ACT.Copy)

            # ---- per m-chunk ----
            for m in range(MCH):
                ms, me = m * 128, (m + 1) * 128
                out_t = res.tile([128, FOUT], f32, tag=f"out{m}")

                # membership mask + n_uniq, C^2 + sum C^2, t2
                mem = res.tile([128, N], bf16, tag=f"mem{m}")
                nuniq = statp.tile([128, 1], f32, tag=f"nu{m}")
                nc.vector.tensor_scalar(out=mem[:], in0=cm_t[m][:], scalar1=1.0,
                                        scalar2=0.0, op0=OP.min, op1=OP.add,
                                        accum_out=nuniq[:])
                cc = junkp.tile([128, N], bf16, tag="cc")
                sumcc = statp.tile([128, 1], f32, tag=f"scc{m}")
                nc.vector.tensor_mul(cc[:], cm_t[m][:], cm_t[m][:])
                nc.vector.tensor_reduce(sumcc[:], cc[:], AX.X, OP.add)
                jk = junkp.tile([128, N], bf16, tag="jk")
                t2 = statp.tile([128, 1], f32, tag=f"t2{m}")
                nc.vector.tensor_mul(jk[:], cc[:], diag_b[:])
                nc.vector.tensor_reduce(t2[:], jk[:], AX.X, OP.add)

                # HR matmul: h_g + a_obs
                phr = phr_pool.tile([128, HRW], f32)
                for k in range(KCH):
                    nc.tensor.matmul(phr[:], ct_t[k][:, ms:me], hr_t[k][:],
                                     start=(k == 0), stop=(k == KCH - 1))
                nc.scalar.activation(out_t[:, 0:D], phr[:, 0:D], ACT.Copy,
                                     scale=1.0 / K)
                aobs = statp.tile([128, 1], f32, tag=f"ao{m}")
                nc.vector.tensor_reduce(aobs[:], phr[:, D:HRW], AX.X, OP.add)
                nc.vector.tensor_scalar_mul(out_t[:, 515:516], aobs[:],
                                            1.0 / (K * NOBS))

                # E matmul: 4 psum banks accumulate over k
                pe_n = []
                for n in range(NCH):
                    pe_n.append(pe_pool.tile([128, 512], f32, tag="pe",
                                             name=f"pe{m}_{n}"))
                for k in range(KCH):
                    for n in range(NCH):
                        nc.tensor.matmul(pe_n[n][:], ct_t[k][:, ms:me],
                                         a_t[k][:, n * 512:(n + 1) * 512],
                                         start=(k == 0), stop=(k == KCH - 1))

                t1p = statp.tile([128, NCH], f32, tag=f"t1p{m}")
                t3p = statp.tile([128, NCH], f32, tag=f"t3p{m}")
                esp = statp.tile([128, NCH], f32, tag=f"esp{m}")
                for n in range(NCH):
                    j1 = junkp.tile([128, 512], bf16, tag="j1")
                    nc.vector.tensor_mul(j1[:], pe_n[n][:],
                                         cm_t[m][:, n * 512:(n + 1) * 512])
                    nc.vector.tensor_reduce(t1p[:, n:n + 1], j1[:], AX.X,
                                            OP.add)
                    j2 = junkp.tile([128, 512], bf16, tag="j2")
                    nc.vector.tensor_mul(j2[:], pe_n[n][:],
                                         mem[:, n * 512:(n + 1) * 512])
                    nc.vector.tensor_reduce(t3p[:, n:n + 1], j2[:], AX.X,
                                            OP.add)
                    nc.vector.tensor_reduce(esp[:, n:n + 1], pe_n[n][:],
                                            AX.X, OP.add)

                # ---- scalar fixups ----
                t1s = statp.tile([128, 1], f32, tag=f"t1s{m}")
                nc.vector.tensor_reduce(t1s[:], t1p[:], AX.X, OP.add)
                t3s = statp.tile([128, 1], f32, tag=f"t3s{m}")
                nc.vector.tensor_reduce(t3s[:], t3p[:], AX.X, OP.add)
                ess = statp.tile([128, 1], f32, tag=f"ess{m}")
                nc.vector.tensor_reduce(ess[:], esp[:], AX.X, OP.add)

                # a_in = (t1 - t2) / max(K*K - sumcc, 1)
                cnt = statp.tile([128, 1], f32, tag=f"cnt{m}")
                nc.vector.tensor_scalar(out=cnt[:], in0=sumcc[:], scalar1=-1.0,
                                        scalar2=float(K * K), op0=OP.mult,
                                        op1=OP.add)
                cntm = statp.tile([128, 1], f32, tag=f"cntm{m}")
                nc.vector.tensor_scalar_max(cntm[:], cnt[:], 1.0)
                rin = statp.tile([128, 1], f32, tag=f"rin{m}")
                nc.vector.reciprocal(rin[:], cntm[:])
                num_in = statp.tile([128, 1], f32, tag=f"ni{m}")
                nc.vector.tensor_sub(num_in[:], t1s[:], t2[:])
                nc.vector.tensor_mul(out_t[:, 513:514], num_in[:], rin[:])

                # a_out = (esum - t3) / (K * (N - nuniq))
                den = statp.tile([128, 1], f32, tag=f"den{m}")
                nc.vector.tensor_scalar(out=den[:], in0=nuniq[:],
                                        scalar1=-float(K),
                                        scalar2=float(K * N), op0=OP.mult,
                                        op1=OP.add)
                rout = statp.tile([128, 1], f32, tag=f"ro{m}")
                nc.vector.reciprocal(rout[:], den[:])
                num_out = statp.tile([128, 1], f32, tag=f"no{m}")
                nc.vector.tensor_sub(num_out[:], ess[:], t3s[:])
                nc.vector.tensor_mul(out_t[:, 514:515], num_out[:], rout[:])

                # ex_dist (mean), ex_clr (min)
                exd = statp.tile([128, 1], f32, tag=f"exd{m}")
                nc.vector.tensor_reduce(exd[:], dg_t[m][:], AX.X, OP.add)
                nc.vector.tensor_scalar_mul(out_t[:, 516:517], exd[:], 1.0 / K)
                nc.vector.tensor_reduce(out_t[:, 517:518], cg_t[m][:], AX.X,
                                        OP.min)

                # h_glob + size_feat
                nc.scalar.activation(out_t[:, D:2 * D], hglob_b[:], ACT.Copy)
                nc.vector.memset(out_t[:, 512:513], float(K) / 3.0)

                nc.sync.dma_start(out=out_d[m * 128:(m + 1) * 128, :],
                                  in_=out_t[:])
    nc.compile()
    return nc


def _get_nc():
    if "nc" not in _NC_CACHE:
        _NC_CACHE["nc"] = _build_nc()
    return _NC_CACHE["nc"]


def kernel(h, attn_rr, attn_ro, dist_to_goal, clearance, groups):
    h = np.asarray(h, dtype=np.float32)
    attn_rr = np.asarray(attn_rr, dtype=np.float32)
    attn_ro = np.asarray(attn_ro, dtype=np.float32)
    dist_to_goal = np.asarray(dist_to_goal, dtype=np.float32)
    clearance = np.asarray(clearance, dtype=np.float32)
    groups = np.asarray(groups)

    a_bf = np.ascontiguousarray(attn_rr.astype(BF16))
    hr_bf = np.ascontiguousarray(
        np.concatenate([h, attn_ro], axis=1).astype(BF16))
    diag_bf = np.ascontiguousarray(np.diagonal(attn_rr)[None, :].astype(BF16))

    in_maps = []
    for s in range(NCORES):
        gs = groups[s * MLOC:(s + 1) * MLOC]
        C = np.zeros((MLOC, N), dtype=np.float32)
        np.add.at(C, (np.arange(MLOC)[:, None], gs), 1.0)
        in_maps.append({
            "a_bf": a_bf,
            "hr_bf": hr_bf,
            "ct_bf": np.ascontiguousarray(C.T.astype(BF16)),
            "cm_bf": np.ascontiguousarray(C.astype(BF16)),
            "diag_bf": diag_bf,
            "cg": np.ascontiguousarray(clearance[gs].astype(np.float32)),
            "dg": np.ascontiguousarray(dist_to_goal[gs].astype(np.float32)),
        })

    nc = _get_nc()
    _NC_CACHE["last_in_maps"] = in_maps
    res = run_bass_kernel_spmd(nc, in_maps, list(range(NCORES)))
    return np.concatenate([res.results[s]["out"] for s in range(NCORES)],
                          axis=0)



# revision 4
# speedup vs baseline: 2.0585x; 2.0585x over previous
"""Trainium2 Bass kernel for nn_GroupFeatureBuilder (segment_reduce).

Strategy: shard the M=4096 groups across 8 cores (512 groups each).
Replace all gathers with dense matmuls against a host-built multiplicity
matrix C[m, n] = (# occurrences of robot n in group m):

  E      = C @ A            (A = attn_rr, fp8e4 DoubleRow)  -> group attn rows
  t1[m]  = <E[m], C[m]>     = sum_{i,j} A[g_i, g_j]
  ao2[m] = <E[m], mem[m]-1> = t3 - esum   (mem = min(C,1))
  a_in   = (t1 - t2) * inv_in        (t2, inv_in from host-gathered smalls)
  a_out  = ao2 * neginv_out
  HR     = C @ [h | rowsum(attn_ro)]  -> h_g (cols 0:256), a_obs (col 256)
  h_glob = ones^T @ h / 2048
  ex_dist/ex_clr/t2: host-gathered (512,16) slots, device reduce.

E is computed in fp8 (A quantization error averages out over the >=256
summands of every stat), copied PSUM->SBUF bf16 by the scalar engine so
both DVE reduction passes run in 2x mode as fused tensor_tensor_reduce.
"""

import numpy as np
import ml_dtypes

import concourse.bass as bass
import concourse.bacc as bacc
import concourse.tile as tile
import concourse.mybir as mybir
from concourse.bass_utils import run_bass_kernel_spmd

BF16 = ml_dtypes.bfloat16
FP8 = ml_dtypes.float8_e4m3

N = 2048       # robots
D = 256        # embed
M = 4096       # groups
K = 16         # group size
NOBS = 64
NCORES = 8
MLOC = M // NCORES     # 512 groups per core
MCH = MLOC // 128      # 4 m-chunks
KCH = N // 128         # 16 contraction chunks
KP = KCH // 2          # 8 DoubleRow k-pairs
HRW = D + 1            # h columns + rowsum(attn_ro) column
FOUT = 2 * D + 6       # 518 output features
SMW = 68               # smalls width

f32 = mybir.dt.float32
bf16 = mybir.dt.bfloat16
fp8 = mybir.dt.float8e4
OP = mybir.AluOpType
AX = mybir.AxisListType
ACT = mybir.ActivationFunctionType
DR = mybir.MatmulPerfMode.DoubleRow

_NC_CACHE = {}


def _build_nc():
    nc = bacc.Bacc("TRN2", target_bir_lowering=False, debug=False,
                   num_devices=NCORES)

    # a_dr rows (j*128+p) hold A[(2j)*128+p, :] | A[(2j+1)*128+p, :]
    a_d = nc.declare_dram_parameter("a_dr", [KP * 128, 2 * N], fp8,
                                    isOutput=False)
    ct_d = nc.declare_dram_parameter("ct_dr", [KP * 128, 2 * MLOC], fp8,
                                     isOutput=False)
    cm_d = nc.declare_dram_parameter("cm_bf", [MLOC, N], bf16, isOutput=False)
    hc_d = nc.declare_dram_parameter("hc_bf", [N, HRW], bf16, isOutput=False)
    sm_d = nc.declare_dram_parameter("sm", [MLOC, SMW], f32, isOutput=False)
    out_d = nc.declare_dram_parameter("out", [MLOC, FOUT], f32, isOutput=True)

    with tile.TileContext(nc) as tc:
        with (
            tc.tile_pool(name="res", bufs=1) as res,
            tc.tile_pool(name="apool", bufs=1) as apool,
            tc.tile_pool(name="esb", bufs=3) as esbp,
            tc.tile_pool(name="junk", bufs=2) as junkp,
            tc.tile_pool(name="om", bufs=2) as omp,
            tc.tile_pool(name="outp", bufs=4) as outp,
            tc.tile_pool(name="stats", bufs=1) as statp,
            tc.tile_pool(name="psum_e", bufs=3, space="PSUM") as pe_pool,
            tc.tile_pool(name="psum_hr", bufs=1, space="PSUM") as phr_pool,
            tc.tile_pool(name="psum_g", bufs=1, space="PSUM") as pg_pool,
        ):
            # ---- resident loads (DMA order = priority order) ----
            ct_t = []
            for j in range(KP):
                t = res.tile([128, 2, MLOC], fp8, tag=f"ct{j}")
                nc.sync.dma_start(out=t[:], in_=ct_d[j * 128:(j + 1) * 128, :])
                ct_t.append(t)
            hc_t = []
            for k in range(KCH):
                t = res.tile([128, HRW], bf16, tag=f"hc{k}")
                nc.sync.dma_start(out=t[:], in_=hc_d[k * 128:(k + 1) * 128, :])
                hc_t.append(t)
            a_t = []
            for j in range(KP):
                t = apool.tile([128, 2, N], fp8, tag=f"a{j}")
                nc.sync.dma_start(out=t[:], in_=a_d[j * 128:(j + 1) * 128, :])
                a_t.append(t)
            cm_t = []
            for m in range(MCH):
                t = res.tile([128, N], bf16, tag=f"cm{m}")
                nc.sync.dma_start(out=t[:], in_=cm_d[m * 128:(m + 1) * 128, :])
                cm_t.append(t)
            sm_t = []
            for m in range(MCH):
                t = res.tile([128, SMW], f32, tag=f"sm{m}")
                nc.sync.dma_start(out=t[:], in_=sm_d[m * 128:(m + 1) * 128, :])
                sm_t.append(t)

            ones_t = res.tile([128, 1], bf16, tag="ones")
            nc.vector.memset(ones_t[:], 1.0)
            ones_row = res.tile([1, 128], bf16, tag="ones_row")
            nc.vector.memset(ones_row[:], 1.0)

            # ---- h_glob = mean_n h[n, :] via ones^T @ h ----
            pg = pg_pool.tile([1, D], f32, tag="pg", name="pg")
            for k in range(KCH):
                nc.tensor.matmul(pg[:], ones_t[:], hc_t[k][:, 0:D],
                                 start=(k == 0), stop=(k == KCH - 1))
            hglob_row = res.tile([1, D], bf16, tag="hglob_row")
            nc.scalar.activation(hglob_row[:], pg[:], ACT.Copy, scale=1.0 / N)
            hglob_b = res.tile([128, D], f32, tag="hglob_b")
            pgb = pg_pool.tile([128, D], f32, tag="pg", name="pgb")
            nc.tensor.matmul(pgb[:], ones_row[:1], hglob_row[:1])
            nc.scalar.activation(hglob_b[:], pgb[:], ACT.Copy)

            # ---- HR matmuls for all m-chunks (h_g + a_obs) ----
            out_t = []
            for m in range(MCH):
                ms, me = m * 128, (m + 1) * 128
                ot = outp.tile([128, FOUT], f32, tag="out", name=f"out{m}")
                out_t.append(ot)
                phr = phr_pool.tile([128, HRW], f32, tag="phr", name=f"phr{m}")
                for j in range(KP):
                    for i in range(2):
                        nc.tensor.matmul(phr[:], ct_t[j][:, i, ms:me],
                                         hc_t[2 * j + i][:],
                                         start=(j == 0 and i == 0),
                                         stop=(j == KP - 1 and i == 1))
                nc.scalar.activation(ot[:, 0:D], phr[:, 0:D], ACT.Copy,
                                     scale=1.0 / K)
                nc.vector.tensor_scalar_mul(ot[:, 515:516], phr[:, D:D + 1],
                                            1.0 / (K * NOBS))

            # ---- per m-chunk: E matmul + fused stat reductions ----
            for m in range(MCH):
                ms, me = m * 128, (m + 1) * 128
                ot = out_t[m]

                # om2 = min(C,1) - 1  (0 inside the group's set, -1 outside)
                om = omp.tile([128, N], bf16, tag="om", name=f"om{m}")
                nc.vector.tensor_scalar(out=om[:], in0=cm_t[m][:],
                                        scalar1=1.0, scalar2=-1.0,
                                        op0=OP.min, op1=OP.add)

                t1h = [statp.tile([128, 1], f32, tag=f"t1{m}_{h}",
                                  name=f"t1{m}_{h}") for h in range(2)]
                aoh = [statp.tile([128, 1], f32, tag=f"ao{m}_{h}",
                                  name=f"ao{m}_{h}") for h in range(2)]
                for hf in range(2):
                    c0 = hf * (N // 2)
                    pe = pe_pool.tile([128, N // 2], f32, tag="pe",
                                      name=f"pe{m}_{hf}")
                    for s in range(2):
                        o0 = s * 512
                        for j in range(KP):
                            nc.tensor.matmul(
                                pe[:, o0:o0 + 512], ct_t[j][:, :, ms:me],
                                a_t[j][:, :, c0 + o0:c0 + o0 + 512],
                                start=(j == 0), stop=(j == KP - 1),
                                perf_mode=DR)
                    esb = esbp.tile([128, N // 2], bf16, tag="esb",
                                    name=f"esb{m}_{hf}")
                    nc.scalar.activation(esb[:], pe[:], ACT.Copy)
                    jk = junkp.tile([128, N // 2], bf16, tag="jk")
                    nc.vector.scalar_tensor_tensor(
                        out=jk[:], in0=esb[:], scalar=1.0,
                        in1=cm_t[m][:, c0:c0 + N // 2],
                        op0=OP.mult, op1=OP.mult, accum_out=t1h[hf][:])
                    jk2 = junkp.tile([128, N // 2], bf16, tag="jk")
                    nc.vector.scalar_tensor_tensor(
                        out=jk2[:], in0=esb[:], scalar=1.0,
                        in1=om[:, c0:c0 + N // 2],
                        op0=OP.mult, op1=OP.mult, accum_out=aoh[hf][:])

                # ---- smalls: t2, ex_dist, ex_clr, a_in, a_out, consts ----
                jks = statp.tile([128, 16], f32, tag=f"jks{m}")
                t2s = statp.tile([128, 1], f32, tag=f"t2s{m}")
                nc.vector.scalar_tensor_tensor(
                    out=jks[:], in0=sm_t[m][:, 32:48], scalar=1.0,
                    in1=sm_t[m][:, 48:64], op0=OP.mult, op1=OP.mult,
                    accum_out=t2s[:])
                exd = statp.tile([128, 1], f32, tag=f"exd{m}")
                nc.vector.tensor_reduce(exd[:], sm_t[m][:, 0:16], AX.X, OP.add)
                nc.vector.tensor_scalar_mul(ot[:, 516:517], exd[:], 1.0 / K)
                nc.vector.tensor_reduce(ot[:, 517:518], sm_t[m][:, 16:32],
                                        AX.X, OP.min)

                # a_in = ((t1h0 + t1h1) - t2) * inv_in, fused two-stage
                t1s = statp.tile([128, 1], f32, tag=f"t1s{m}")
                nc.vector.tensor_sub(t1s[:], t1h[0][:], t2s[:])
                nc.vector.scalar_tensor_tensor(
                    out=ot[:, 513:514], in0=t1s[:], scalar=t1h[1][:],
                    in1=sm_t[m][:, 65:66], op0=OP.add, op1=OP.mult)
                # a_out = (aoh0 + aoh1) * neginv_out
                nc.vector.scalar_tensor_tensor(
                    out=ot[:, 514:515], in0=aoh[0][:], scalar=aoh[1][:],
                    in1=sm_t[m][:, 64:65], op0=OP.add, op1=OP.mult)
                nc.vector.memset(ot[:, 512:513], float(K) / 3.0)
                nc.scalar.activation(ot[:, D:2 * D], hglob_b[:], ACT.Copy)

                nc.sync.dma_start(out=out_d[m * 128:(m + 1) * 128, :],
                                  in_=ot[:])
    nc.compile()
    return nc


def _get_nc():
    if "nc" not in _NC_CACHE:
        _NC_CACHE["nc"] = _build_nc()
    return _NC_CACHE["nc"]


def kernel(h, attn_rr, attn_ro, dist_to_goal, clearance, groups):
    h = np.asarray(h, dtype=np.float32)
    attn_rr = np.asarray(attn_rr, dtype=np.float32)
    attn_ro = np.asarray(attn_ro, dtype=np.float32)
    dist_to_goal = np.asarray(dist_to_goal, dtype=np.float32)
    clearance = np.asarray(clearance, dtype=np.float32)
    groups = np.asarray(groups)

    # DoubleRow pair layout: row (j*128+p) = [X[2j*128+p, :] | X[(2j+1)*128+p, :]]
    def dr_pairs(x):
        w = x.shape[1]
        return np.ascontiguousarray(
            x.reshape(KP, 2, 128, w).transpose(0, 2, 1, 3).reshape(KP * 128,
                                                                   2 * w))

    a_dr = dr_pairs(attn_rr.astype(FP8))
    robs = attn_ro.sum(axis=1, dtype=np.float32)
    hc_bf = np.ascontiguousarray(
        np.concatenate([h, robs[:, None]], axis=1).astype(BF16))
    diag = np.ascontiguousarray(np.diagonal(attn_rr)).astype(np.float32)

    in_maps = []
    for s in range(NCORES):
        gs = groups[s * MLOC:(s + 1) * MLOC]
        C = np.zeros((MLOC, N), dtype=np.float32)
        np.add.at(C, (np.arange(MLOC)[:, None], gs), 1.0)

        sumcc = (C * C).sum(axis=1)
        nuniq = (C > 0).sum(axis=1).astype(np.float32)
        sm = np.zeros((MLOC, SMW), dtype=np.float32)
        sm[:, 0:16] = dist_to_goal[gs]
        sm[:, 16:32] = clearance[gs]
        sm[:, 32:48] = diag[gs]
        sm[:, 48:64] = C[np.arange(MLOC)[:, None], gs]
        sm[:, 64] = -1.0 / (K * (N - nuniq))
        sm[:, 65] = 1.0 / np.maximum(K * K - sumcc, 1.0)

        in_maps.append({
            "a_dr": a_dr,
            "ct_dr": dr_pairs(C.T.astype(FP8)),
            "cm_bf": np.ascontiguousarray(C.astype(BF16)),
            "hc_bf": hc_bf,
            "sm": sm,
        })

    nc = _get_nc()
    _NC_CACHE["last_in_maps"] = in_maps
    res = run_bass_kernel_spmd(nc, in_maps, list(range(NCORES)))
    return np.concatenate([res.results[s]["out"] for s in range(NCORES)],
                          axis=0)


# revision 9
# speedup vs baseline: 2.2596x; 1.0977x over previous
"""Trainium2 Bass kernel for nn_GroupFeatureBuilder (segment_reduce).

Strategy: shard the M=4096 groups across 8 cores (512 groups each).
Replace all gathers with dense matmuls against a host-built multiplicity
matrix C[m, n] = (# occurrences of robot n in group m).

X = [attn_rr | h_hi | h_lo] in fp8e4, DoubleRow pair layout.  Per m-chunk
one j-loop of DoubleRow matmuls computes E = C@A (banks 0..3) and
h_g hi/lo (bank 4) with a shared stationary ct slice:

  t1[m]  = <E[m], C[m]>          (fused DVE pass, PSUM direct)
  ao2[m] = <E[m], min(C,1)-1>    = t3 - esum
  a_in   = (t1 - t2) * inv_in    (t2, inv_in from host-gathered smalls)
  a_out  = ao2 * neginv_out
  h_g    = (hi_g + lo_g/16)/16   (hi = fp8(h), lo = fp8((h-hi)*16))
  h_glob via ones^T pair-matmuls on the same hi/lo columns
  a_obs/ex_dist/ex_clr/t2: host-gathered (512,16) slots, device reduce.

fp8 quantization error on A averages out over the >=256 summands of every
attention stat; h keeps ~bf16 accuracy through the hi/lo split.
"""

import numpy as np
import ml_dtypes

import concourse.bass as bass
import concourse.bacc as bacc
import concourse.tile as tile
import concourse.mybir as mybir
from concourse.bass_utils import run_bass_kernel_spmd

BF16 = ml_dtypes.bfloat16
FP8 = ml_dtypes.float8_e4m3

N = 2048       # robots
D = 256        # embed
M = 4096       # groups
K = 16         # group size
NOBS = 64
NCORES = 8
MLOC = M // NCORES     # 512 groups per core
MCH = MLOC // 128      # 4 m-chunks
KP = 8                 # DoubleRow k-pairs (16 chunks of 128)
XW = N + 2 * D         # 2560 columns of X = [A | h_hi | h_lo]
FOUT = 2 * D + 6       # 518 output features
SMW = 84               # smalls width

f32 = mybir.dt.float32
bf16 = mybir.dt.bfloat16
fp8 = mybir.dt.float8e4
OP = mybir.AluOpType
AX = mybir.AxisListType
ACT = mybir.ActivationFunctionType
DR = mybir.MatmulPerfMode.DoubleRow

_NC_CACHE = {}


def _build_nc():
    nc = bacc.Bacc("TRN2", target_bir_lowering=False, debug=False,
                   num_devices=NCORES)

    # All host layouts are partition-major: row p holds that partition's
    # bytes.  a/ct pair layout: [p, j, i, c] = X[(2j+i)*128+p, c].
    a_d = nc.declare_dram_parameter("a_x", [128, KP * 2 * XW], fp8,
                                    isOutput=False)
    ct_d = nc.declare_dram_parameter("ct_x", [128, KP * 2 * MLOC], fp8,
                                     isOutput=False)
    cm_d = nc.declare_dram_parameter("cm_x", [128, MCH * N], bf16,
                                     isOutput=False)
    sm_d = nc.declare_dram_parameter("sm_x", [128, MCH * SMW], f32,
                                     isOutput=False)
    out_d = nc.declare_dram_parameter("out", [MLOC, FOUT], f32, isOutput=True)

    with tile.TileContext(nc) as tc:
        with (
            tc.tile_pool(name="res", bufs=1) as res,
            tc.tile_pool(name="om", bufs=2) as omp,
            tc.tile_pool(name="junk", bufs=2) as junkp,
            tc.tile_pool(name="outp", bufs=2) as outp,
            tc.tile_pool(name="stats", bufs=1) as statp,
            tc.tile_pool(name="psum_a", bufs=3, space="PSUM") as peA_pool,
            tc.tile_pool(name="psum_h", bufs=1, space="PSUM") as peH_pool,
            tc.tile_pool(name="psum_g", bufs=1, space="PSUM") as pg_pool,
        ):
            # ---- resident tiles + fused DMAs (order = priority) ----
            ct_all = res.tile([128, KP, 2, MLOC], fp8, tag="ct_all")
            nc.sync.dma_start(out=ct_all[:, 0:4, :, :],
                              in_=ct_d[:, 0:4 * 2 * MLOC])
            a_all = res.tile([128, KP, 2, XW], fp8, tag="a_all")
            nc.sync.dma_start(out=a_all[:, 0, :, :], in_=a_d[:, 0:2 * XW])
            nc.sync.dma_start(out=ct_all[:, 4:8, :, :],
                              in_=ct_d[:, 4 * 2 * MLOC:])
            cm_all = res.tile([128, MCH, N], bf16, tag="cm_all")
            nc.sync.dma_start(out=cm_all[:, 0, :], in_=cm_d[:, 0:N])
            for j in range(1, KP):
                nc.sync.dma_start(out=a_all[:, j, :, :],
                                  in_=a_d[:, j * 2 * XW:(j + 1) * 2 * XW])
            nc.sync.dma_start(out=cm_all[:, 1:MCH, :], in_=cm_d[:, N:])
            sm_all = res.tile([128, MCH, SMW], f32, tag="sm_all")
            nc.sync.dma_start(out=sm_all[:], in_=sm_d[:, :])

            ones_p = res.tile([128, 2, 1], fp8, tag="ones_p")
            nc.vector.memset(ones_p[:], 1.0)
            ones_row = res.tile([1, 128], bf16, tag="ones_row")
            nc.vector.memset(ones_row[:], 1.0)

            hglob_b = res.tile([128, D], f32, tag="hglob_b")

            for m in range(MCH):
                ms, me = m * 128, (m + 1) * 128
                ot = outp.tile([128, FOUT], f32, tag="out", name=f"out{m}")

                # ---- unit-major DoubleRow matmuls, shared ct slice per j ----
                peA = [peA_pool.tile([128, 1024], f32, tag="peA",
                                     name=f"peA{m}_{h}") for h in range(2)]
                for h in range(2):
                    for j in range(KP):
                        lhs = ct_all[:, j, :, ms:me]
                        for s in range(2):
                            c0 = h * 1024 + s * 512
                            nc.tensor.matmul(
                                peA[h][:, s * 512:(s + 1) * 512], lhs,
                                a_all[:, j, :, c0:c0 + 512],
                                start=(j == 0), stop=(j == KP - 1),
                                perf_mode=DR)
                peH = peH_pool.tile([128, 512], f32, tag="peH",
                                    name=f"peH{m}")
                for j in range(KP):
                    nc.tensor.matmul(peH[:], ct_all[:, j, :, ms:me],
                                     a_all[:, j, :, N:XW],
                                     start=(j == 0), stop=(j == KP - 1),
                                     perf_mode=DR)
                if m == 0:
                    # h_glob: ones^T over the hi/lo columns (plain fp8)
                    pg = pg_pool.tile([1, 512], f32, tag="pg", name="pg")
                    for j in range(KP):
                        for i in range(2):
                            nc.tensor.matmul(
                                pg[:], ones_p[:, i, 0:1],
                                a_all[:, j, i, N:XW],
                                start=(j == 0 and i == 0),
                                stop=(j == KP - 1 and i == 1))
                    hgfs = statp.tile([1, D], f32, tag="hgfs")
                    nc.scalar.activation(hgfs[:], pg[:, 0:D], ACT.Copy)
                    hgf = statp.tile([1, D], f32, tag="hgf")
                    nc.vector.scalar_tensor_tensor(
                        out=hgf[:], in0=pg[:, D:2 * D], scalar=1.0 / 16.0,
                        in1=hgfs[:], op0=OP.mult, op1=OP.add)
                    hgrow = statp.tile([1, D], bf16, tag="hgrow")
                    nc.scalar.activation(hgrow[:], hgf[:], ACT.Copy,
                                         scale=1.0 / N)
                    pgb = pg_pool.tile([128, D], f32, tag="pg", name="pgb")
                    nc.tensor.matmul(pgb[:], ones_row[:1], hgrow[:1])
                    nc.scalar.activation(hglob_b[:], pgb[:], ACT.Copy)

                # ---- DVE drains: om mask, fused E reductions (PSUM direct) --
                om = omp.tile([128, N], bf16, tag="om", name=f"om{m}")
                nc.vector.tensor_scalar(out=om[:], in0=cm_all[:, m, :],
                                        scalar1=1.0, scalar2=-1.0,
                                        op0=OP.min, op1=OP.add)
                t1h = [statp.tile([128, 1], f32, tag=f"t1{m}_{h}",
                                  name=f"t1{m}_{h}") for h in range(2)]
                aoh = [statp.tile([128, 1], f32, tag=f"ao{m}_{h}",
                                  name=f"ao{m}_{h}") for h in range(2)]
                for h in range(2):
                    c0 = h * 1024
                    jk = junkp.tile([128, 1024], bf16, tag="jk")
                    nc.vector.scalar_tensor_tensor(
                        out=jk[:], in0=peA[h][:], scalar=1.0,
                        in1=cm_all[:, m, c0:c0 + 1024],
                        op0=OP.mult, op1=OP.mult, accum_out=t1h[h][:])
                    jk2 = junkp.tile([128, 1024], bf16, tag="jk")
                    nc.vector.scalar_tensor_tensor(
                        out=jk2[:], in0=peA[h][:], scalar=1.0,
                        in1=om[:, c0:c0 + 1024],
                        op0=OP.mult, op1=OP.mult, accum_out=aoh[h][:])

                # h_g = hi_g/16 + lo_g/256
                hgs = statp.tile([128, D], f32, tag=f"hgs{m}")
                nc.scalar.activation(hgs[:], peH[:, 0:D], ACT.Copy,
                                     scale=1.0 / K)
                nc.vector.scalar_tensor_tensor(
                    out=ot[:, 0:D], in0=peH[:, D:2 * D],
                    scalar=1.0 / (K * 16.0), in1=hgs[:],
                    op0=OP.mult, op1=OP.add)

                # ---- smalls ----
                jks = statp.tile([128, 16], f32, tag=f"jks{m}")
                t2s = statp.tile([128, 1], f32, tag=f"t2s{m}")
                nc.vector.scalar_tensor_tensor(
                    out=jks[:], in0=sm_all[:, m, 32:48], scalar=1.0,
                    in1=sm_all[:, m, 48:64], op0=OP.mult, op1=OP.mult,
                    accum_out=t2s[:])
                exd = statp.tile([128, 1], f32, tag=f"exd{m}")
                nc.vector.tensor_reduce(exd[:], sm_all[:, m, 0:16], AX.X,
                                        OP.add)
                nc.vector.tensor_scalar_mul(ot[:, 516:517], exd[:], 1.0 / K)
                nc.vector.tensor_reduce(ot[:, 517:518], sm_all[:, m, 16:32],
                                        AX.X, OP.min)
                aob = statp.tile([128, 1], f32, tag=f"aob{m}")
                nc.vector.tensor_reduce(aob[:], sm_all[:, m, 64:80], AX.X,
                                        OP.add)
                nc.vector.tensor_scalar_mul(ot[:, 515:516], aob[:],
                                            1.0 / (K * NOBS))

                # a_in = ((t1h0 + t1h1) - t2) * inv_in
                t1s = statp.tile([128, 1], f32, tag=f"t1s{m}")
                nc.vector.tensor_sub(t1s[:], t1h[0][:], t2s[:])
                nc.vector.scalar_tensor_tensor(
                    out=ot[:, 513:514], in0=t1s[:], scalar=t1h[1][:],
                    in1=sm_all[:, m, 81:82], op0=OP.add, op1=OP.mult)
                # a_out = (aoh0 + aoh1) * neginv_out
                nc.vector.scalar_tensor_tensor(
                    out=ot[:, 514:515], in0=aoh[0][:], scalar=aoh[1][:],
                    in1=sm_all[:, m, 80:81], op0=OP.add, op1=OP.mult)
                nc.vector.memset(ot[:, 512:513], float(K) / 3.0)
                nc.scalar.activation(ot[:, D:2 * D], hglob_b[:], ACT.Copy)

                nc.sync.dma_start(out=out_d[m * 128:(m + 1) * 128, :],
                                  in_=ot[:])
    nc.compile()
    return nc


def _get_nc():
    if "nc" not in _NC_CACHE:
        _NC_CACHE["nc"] = _build_nc()
    return _NC_CACHE["nc"]


def _pair_layout(x):
    """[2048, w] -> [128, 8*2*w]: row p holds [X[2j*128+p,:] | X[(2j+1)*128+p,:]] per j."""
    w = x.shape[1]
    return np.ascontiguousarray(
        x.reshape(KP, 2, 128, w).transpose(2, 0, 1, 3).reshape(128,
                                                               KP * 2 * w))


def kernel(h, attn_rr, attn_ro, dist_to_goal, clearance, groups):
    h = np.asarray(h, dtype=np.float32)
    attn_rr = np.asarray(attn_rr, dtype=np.float32)
    attn_ro = np.asarray(attn_ro, dtype=np.float32)
    dist_to_goal = np.asarray(dist_to_goal, dtype=np.float32)
    clearance = np.asarray(clearance, dtype=np.float32)
    groups = np.asarray(groups)

    h_hi = h.astype(FP8)
    h_lo = ((h - h_hi.astype(np.float32)) * 16.0).astype(FP8)
    x_full = np.concatenate(
        [attn_rr.astype(FP8), h_hi, h_lo], axis=1)          # [2048, 2560] fp8
    a_x = _pair_layout(x_full)
    robs = attn_ro.sum(axis=1, dtype=np.float32)
    diag = np.ascontiguousarray(np.diagonal(attn_rr)).astype(np.float32)

    in_maps = []
    for s in range(NCORES):
        gs = groups[s * MLOC:(s + 1) * MLOC]
        C = np.zeros((MLOC, N), dtype=np.float32)
        np.add.at(C, (np.arange(MLOC)[:, None], gs), 1.0)

        sumcc = (C * C).sum(axis=1)
        nuniq = (C > 0).sum(axis=1).astype(np.float32)
        sm = np.zeros((MLOC, SMW), dtype=np.float32)
        sm[:, 0:16] = dist_to_goal[gs]
        sm[:, 16:32] = clearance[gs]
        sm[:, 32:48] = diag[gs]
        sm[:, 48:64] = C[np.arange(MLOC)[:, None], gs]
        sm[:, 64:80] = robs[gs]
        sm[:, 80] = -1.0 / (K * (N - nuniq))
        sm[:, 81] = 1.0 / np.maximum(K * K - sumcc, 1.0)

        in_maps.append({
            "a_x": a_x,
            "ct_x": _pair_layout(C.T.astype(FP8)),
            "cm_x": np.ascontiguousarray(
                C.astype(BF16).reshape(MCH, 128, N).transpose(1, 0, 2)
                .reshape(128, MCH * N)),
            "sm_x": np.ascontiguousarray(
                sm.reshape(MCH, 128, SMW).transpose(1, 0, 2)
                .reshape(128, MCH * SMW)),
        })

    nc = _get_nc()
    _NC_CACHE["last_in_maps"] = in_maps
    res = run_bass_kernel_spmd(nc, in_maps, list(range(NCORES)))
    return np.concatenate([res.results[s]["out"] for s in range(NCORES)],
                          axis=0)
